# revision 3
# baseline (speedup 1.0000x reference)
"""Distributed Bass kernel for nn_Attention (B=2, S=2048, HID=2048, H=32, KVH=8, D=64).

Sharding (8 NeuronCores, uniform SPMD graph):
  - Head-parallel phase: core c owns kv-head c and its 4 GQA query heads.
    x replicated as xT [HID, B*S] bf16; per-core Q^T [256, 4096] (+RoPE,
    1/sqrt(D) folded into trig tables), K^T [64, 4096] (+RoPE, duplicated to
    rows 64:128), V [keys, 64|1] blocks with a ones-column for the softmax
    denominator.
  - Attention in S^T layout [keys, queries], processed per head-PAIR: the two
    heads of a pair sit on PE row-groups 0/64, so their K=64 S^T matmuls run
    concurrently (row-group tiling).  exp is split between the Scalar engine
    (true Exp) and the Vector engine (one-instruction Schraudolph bf16
    bit-trick exp ~ bitcast(int16(x*128/ln2 + 16248.75))) so neither engine
    bottlenecks.  Causal masking: rectangular blocks need no mask; the 4
    diagonal-band blocks per chunk use width-trimmed matmuls plus a 0/1 mask
    multiply (split Vector/GpSimd).
  - Per-unit drains DMA straight into the AllToAll staging buffer
    [NC, 130, TL] whose rows 64/129 carry the (bf16) softmax denominators —
    one collective per head-pair, no separate denominator collective.  Pair 0's
    collective overlaps pair 1's attention.
  - Token-parallel output projection: ao tiles normalized post-collective,
    out rows = ao.T @ wo per 512-col block; wo prefetched during attention.
"""

import os
import numpy as np
import ml_dtypes

import concourse.bass as bass
import concourse.mybir as mybir
import concourse.tile as tile
from concourse import bacc
from concourse.bass_utils import run_bass_kernel_spmd

BF16 = ml_dtypes.bfloat16
F32 = np.float32

B, S, HID = 2, 2048, 2048
H, KVH, D = 32, 8, 64
NC = 8                 # cores
T = B * S              # 4096 flat tokens
TL = T // NC           # 512 tokens per core (phase-2 output rows)
LH = H // NC           # 4 local q-heads per core
KB = 128               # key block
TC = 512               # phase-1 token streaming chunk / query chunk
NTC = T // TC          # 8 token chunks

SCHR_A = float(128.0 / np.log(2.0))
SCHR_B = float(127.0 * 128.0 - 7.25)

_CACHE = {}


def _build():
    fp32 = mybir.dt.float32
    bf16 = mybir.dt.bfloat16
    i16 = mybir.dt.int16

    # knobs (read at build time; defaults are the tuned config)
    DVE_EXP = float(os.environ.get("KDVE", "0.5"))   # fraction of exps on DVE
    DIAG_TRIM = os.environ.get("KDIAG", "1") == "1"  # width-trim diagonal blocks

    nc = bacc.Bacc("TRN2", target_bir_lowering=False, debug=False, num_devices=NC)

    xT = nc.dram_tensor("xT", [HID, T], bf16, kind="ExternalInput")
    wq_c = nc.dram_tensor("wq_c", [HID, LH * D], bf16, kind="ExternalInput")
    wkv_c = nc.dram_tensor("wkv_c", [HID, 2 * D], bf16, kind="ExternalInput")
    wo = nc.dram_tensor("wo", [HID, HID], bf16, kind="ExternalInput")
    ctq_d = nc.dram_tensor("ctq", [128, 2, T], fp32, kind="ExternalInput")
    ctk_d = nc.dram_tensor("ctk", [64, 2, T], fp32, kind="ExternalInput")
    mq_d = nc.dram_tensor("mq", [128, 4, 2 * TC], bf16, kind="ExternalInput")
    out_d = nc.dram_tensor("out", [TL, HID], fp32, kind="ExternalOutput")

    with tile.TileContext(nc) as tc:
        with (
            tc.tile_pool(name="persist", bufs=1) as persist,
            tc.tile_pool(name="stream", bufs=2) as stream,
            tc.tile_pool(name="trig", bufs=2) as trig,
            tc.tile_pool(name="work", bufs=2) as work,
            tc.tile_pool(name="psum", bufs=1, space="PSUM") as psum,
            tc.tile_pool(name="dram", bufs=1, space="DRAM") as dram,
        ):
            # ---- persistent tiles ----
            qT = [persist.tile([128, T], bf16, tag=f"qT{t}", name=f"qT{t}")
                  for t in range(2)]
            k2 = persist.tile([128, T], bf16, tag="k2", name="k2")
            vatt = [persist.tile([128, D + 1], bf16, tag=f"vatt{i}", name=f"vatt{i}")
                    for i in range(T // KB)]
            mq = persist.tile([128, 4, 2 * TC], bf16, tag="mq", name="mq")
            ident = persist.tile([128, 128], bf16, tag="ident", name="ident")

            # ---- weights (merged single-DMA loads) ----
            wq_sb = persist.tile([128, 16, LH * D], bf16, tag="wq", name="wq")
            wkv_sb = persist.tile([128, 16, 2 * D], bf16, tag="wkv", name="wkv")
            nc.sync.dma_start(wq_sb[:], wq_c.rearrange("(k p) m -> p k m", p=128))
            nc.scalar.dma_start(wkv_sb[:], wkv_c.rearrange("(k p) m -> p k m", p=128))

            # gpsimd queue: masks + ones-columns + identity (not startup-critical)
            nc.gpsimd.dma_start(mq[:], mq_d[:])
            for i in range(T // KB):
                nc.gpsimd.memset(vatt[i][:, D:D + 1], 1.0)
            from concourse.masks import make_identity
            make_identity(nc, ident[:])

            xT_r = xT.rearrange("(k p) t -> p k t", p=128)

            # ================= Phase 1: QKV projections + RoPE =================
            def rope(out_ap, ps, ct, st, npart):
                """out = ps*ct + swap32(ps)*st  (st carries the rotate-half sign)."""
                t1 = work.tile([128, TC], fp32, tag="rope_t1", name="t1")
                t2 = work.tile([128, TC], fp32, tag="rope_t2", name="t2")
                nc.vector.tensor_mul(t1[:npart, :], ps[:npart, :], ct[:npart, :])
                for base in range(0, npart, 64):
                    a, b2 = base, base + 32
                    nc.vector.tensor_mul(t2[a:a + 32, :], ps[b2:b2 + 32, :], st[a:a + 32, :])
                    nc.vector.tensor_mul(t2[b2:b2 + 32, :], ps[a:a + 32, :], st[b2:b2 + 32, :])
                nc.vector.tensor_add(out_ap, t1[:npart, :], t2[:npart, :])

            for tc8 in range(NTC):
                tsl = slice(TC * tc8, TC * (tc8 + 1))
                xt = stream.tile([128, 16, TC], bf16, tag="s", name=f"x{tc8}")
                nc.sync.dma_start(xt[:], xT_r[:, :, tsl])
                ctq = trig.tile([128, 2, TC], fp32, tag="ctq", name="ctq")
                ctk = trig.tile([64, 2, TC], fp32, tag="ctk", name="ctk")
                nc.scalar.dma_start(ctq[:], ctq_d[:, :, tsl])
                nc.scalar.dma_start(ctk[:], ctk_d[:, :, tsl])

                # Q^T: two 128-row tiles (2 heads each)
                for qt in range(2):
                    ps = psum.tile([128, 2 * TC], fp32, tag="big", bufs=3,
                                   name="ps_q")[:, 0:TC]
                    for k in range(16):
                        nc.tensor.matmul(ps[:], wq_sb[:, k, 128 * qt:128 * (qt + 1)],
                                         xt[:, k, :], start=(k == 0), stop=(k == 15))
                    rope(qT[qt][:, tsl], ps, ctq[:, 0, :], ctq[:, 1, :], 128)

                # K^T (rows 0:64) and V^T (rows 64:128) in one packed projection
                ps = psum.tile([128, 2 * TC], fp32, tag="big", bufs=3,
                               name="ps_kv")[:, 0:TC]
                for k in range(16):
                    nc.tensor.matmul(ps[:], wkv_sb[:, k, :], xt[:, k, :],
                                     start=(k == 0), stop=(k == 15))
                rope(k2[0:64, tsl], ps, ctk[:, 0, :], ctk[:, 1, :], 64)
                nc.scalar.copy(k2[64:128, tsl], k2[0:64, tsl])

                vt = work.tile([64, TC], bf16, tag="vt", name="vt")
                nc.scalar.copy(vt[:], ps[64:128, :])
                for j in range(TC // KB):
                    kbi = (TC // KB) * tc8 + j
                    pst = psum.tile([128, TC], bf16, tag="mm", bufs=2, name="ps_tr")
                    nc.tensor.transpose(pst[:, 0:64], vt[:, 128 * j:128 * (j + 1)],
                                        ident[0:64, 0:64])
                    nc.vector.tensor_copy(vatt[kbi][:, 0:D], pst[:, 0:64])

            # ---- wo prefetch: runs on DMA queues during attention ----
            # nt 0/1 get persistent tiles; nt 2/3 reuse the freed x-stream
            # slots (same [128, 16, 512] bf16 shape, x is done after phase 1).
            wo_r = wo.rearrange("(k p) n -> p k n", p=128)
            wo_sb = []
            w_eng = (nc.sync, nc.scalar, nc.gpsimd, nc.sync)
            for nt in range(4):
                if nt < 2:
                    t = persist.tile([128, 16, 512], bf16, tag=f"wo{nt}",
                                     name=f"wo{nt}")
                else:
                    t = stream.tile([128, 16, 512], bf16, tag="s", name=f"wo{nt}")
                w_eng[nt].dma_start(t[:], wo_r[:, :, 512 * nt:512 * (nt + 1)])
                wo_sb.append(t)

            # ================= Attention (head-pair parallel, causal) ==========
            a2a_in = [dram.tile([NC, 130, TL], bf16, tag=f"a2a_in{t}",
                                name=f"a2a_in{t}") for t in range(2)]
            a2a_out = [dram.tile([NC, 130, TL], bf16, tag=f"a2a_out{t}",
                                 name=f"a2a_out{t}") for t in range(2)]
            rstage = [dram.tile([2 * NC, TL], fp32, tag=f"rstage{t}",
                                name=f"rstage{t}") for t in range(2)]
            ao = {}
            exp_ctr = 0
            mask_ctr = 0
            exp_acc = 0.0

            for pair in range(2):
                qtile = qT[pair]
                for b in range(B):
                    for cq in range(4):
                        nkb = 4 * (cq + 1)
                        qs = S * b + TC * cq
                        j = 4 * b + cq          # destination core for this unit
                        psOa = psum.tile([128, 512], fp32, tag="mm", bufs=2,
                                         name="psOa")[0:D + 1, :]
                        psOb = psum.tile([128, 512], fp32, tag="mm", bufs=2,
                                         name="psOb")[0:D + 1, :]
                        for kb in range(nkb):
                            dj = kb - (nkb - 4)  # >=0: diagonal-band index
                            qoff = 128 * dj if (DIAG_TRIM and dj > 0) else 0
                            kpos = S * b + KB * kb
                            psS = psum.tile([128, 2 * TC], fp32, tag="big",
                                            bufs=3, name="psS")
                            ex = work.tile([128, 2 * TC], bf16, tag="ex", bufs=6,
                                           name="ex")
                            for h in range(2):
                                nc.tensor.matmul(
                                    psS[:, TC * h + qoff:TC * (h + 1)],
                                    k2[64 * h:64 * (h + 1), kpos:kpos + KB],
                                    qtile[64 * h:64 * (h + 1),
                                          qs + qoff:qs + TC],
                                    start=True, stop=True)
                            ps2 = psS.rearrange("p (h c) -> p h c", h=2)
                            ex2 = ex.rearrange("p (h c) -> p h c", h=2)
                            exi = ex.bitcast(i16).rearrange("p (h c) -> p h c", h=2)
                            exp_acc += DVE_EXP
                            if exp_acc >= 1.0:
                                exp_acc -= 1.0
                                nc.vector.tensor_scalar(
                                    exi[:, :, qoff:TC], ps2[:, :, qoff:TC],
                                    SCHR_A, SCHR_B,
                                    mybir.AluOpType.mult, mybir.AluOpType.add)
                            else:
                                nc.scalar.activation(
                                    ex2[:, :, qoff:TC], ps2[:, :, qoff:TC],
                                    mybir.ActivationFunctionType.Exp)
                            exp_ctr += 1
                            if dj >= 0:
                                m2 = mq[:, dj, :].rearrange("p (h c) -> p h c", h=2)
                                meng = nc.gpsimd if (mask_ctr % 2 == 0) else nc.vector
                                mask_ctr += 1
                                meng.tensor_mul(ex2[:, :, qoff:TC],
                                                ex2[:, :, qoff:TC],
                                                m2[:, :, qoff:TC])
                            for h, psO in ((0, psOa), (1, psOb)):
                                nc.tensor.matmul(
                                    psO[:, qoff:TC],
                                    vatt[(S // KB) * b + kb][:],
                                    ex2[:, h, qoff:TC],
                                    start=(kb == 0), stop=(kb == nkb - 1))
                        # drain: [attn 64 rows | den row] straight to a2a staging
                        for h, psO in ((0, psOa), (1, psOb)):
                            bounce = work.tile([D + 1, 512], bf16, tag="bounce",
                                               bufs=4, name="bounce")
                            deng = nc.scalar if (h == 0) else nc.vector
                            if deng is nc.scalar:
                                nc.scalar.copy(bounce[:], psO[:])
                            else:
                                nc.vector.tensor_copy(bounce[:], psO[:])
                            nc.sync.dma_start(
                                a2a_in[pair][j, 65 * h:65 * (h + 1), :], bounce[:])

                # -------- collective + normalization for this pair ------------
                nc.gpsimd.collective_compute(
                    "AllToAll", mybir.AluOpType.bypass,
                    replica_groups=[list(range(NC))],
                    ins=[a2a_in[pair].opt()], outs=[a2a_out[pair].opt()])
                den_all = work.tile([2 * NC, TL], bf16, tag="den_all", bufs=2,
                                    name="den_all")
                a2a_v = a2a_out[pair].rearrange("a (x b) c -> (a x) b c", x=2)
                nc.sync.dma_start(den_all[:], a2a_v[:, 64, :])
                rall = work.tile([2 * NC, TL], fp32, tag="rall", bufs=2,
                                 name="rall")
                nc.vector.reciprocal(rall[:], den_all[:])
                nc.sync.dma_start(rstage[pair][:], rall[:])
                for r in range(NC):
                    kk = 2 * r + pair
                    tl_ = persist.tile([128, TL], bf16, tag=f"ao{kk}",
                                       name=f"ao{kk}")
                    nc.sync.dma_start(tl_[0:64, :], a2a_out[pair][r, 0:64, :])
                    nc.sync.dma_start(tl_[64:128, :], a2a_out[pair][r, 65:129, :])
                    rb2 = work.tile([128, TL], fp32, tag="rb2", bufs=2,
                                    name="rb2")
                    beng = nc.gpsimd if (r % 2 == 0) else nc.sync
                    beng.dma_start(
                        rb2[0:64, :],
                        rstage[pair][2 * r:2 * r + 1, :].broadcast_to([64, TL]))
                    beng.dma_start(
                        rb2[64:128, :],
                        rstage[pair][2 * r + 1:2 * r + 2, :].broadcast_to([64, TL]))
                    nc.vector.tensor_mul(tl_[:], tl_[:], rb2[:])
                    ao[kk] = tl_

            # ================= Phase 2: output projection =====================
            for nt in range(4):
                for tt in range(TL // 128):
                    ps = psum.tile([128, 512], fp32, tag="mm", bufs=2, name="ps_o")
                    for kk in range(16):
                        nc.tensor.matmul(ps[:], ao[kk][:, 128 * tt:128 * (tt + 1)],
                                         wo_sb[nt][:, kk, :],
                                         start=(kk == 0), stop=(kk == 15))
                    ob = work.tile([128, 512], fp32, tag="ob", bufs=2, name="ob")
                    if (nt + tt) % 2 == 0:
                        nc.scalar.copy(ob[:], ps[:])
                    else:
                        nc.vector.tensor_copy(ob[:], ps[:])
                    oeng = nc.gpsimd if (nt + tt) % 2 == 0 else nc.sync
                    oeng.dma_start(out_d[128 * tt:128 * (tt + 1),
                                         512 * nt:512 * (nt + 1)], ob[:])

    nc.compile()
    return nc


def _prep_inputs(x, cos, sin, wq, wk, wv, wo):
    x = np.asarray(x, F32)
    cos = np.asarray(cos, F32)
    sin = np.asarray(sin, F32)
    wq = np.asarray(wq, F32)
    wk = np.asarray(wk, F32)
    wv = np.asarray(wv, F32)
    wo = np.asarray(wo, F32)

    xT = np.ascontiguousarray(x.reshape(T, HID).T).astype(BF16)
    wo_b = wo.astype(BF16)

    pos = np.arange(T) % S
    sign = np.concatenate([-np.ones(D // 2, F32), np.ones(D // 2, F32)])
    ctk = np.ascontiguousarray(cos[pos].T)                      # [64, T]
    stk = np.ascontiguousarray((sin[pos] * sign).T)             # [64, T]
    ctk2 = np.ascontiguousarray(np.stack([ctk, stk], 1))        # [64, 2, T]
    scale = F32(1.0 / np.sqrt(D))
    ctq2 = np.ascontiguousarray(
        np.stack([np.concatenate([ctk, ctk], 0) * scale,
                  np.concatenate([stk, stk], 0) * scale], 1))   # [128, 2, T]

    qlw = np.arange(TC)
    kl = np.arange(128)
    mas = [(qlw[None, :] >= (kl[:, None] + 128 * jj)).astype(BF16)
           for jj in range(4)]                     # 512-wide diagonal masks
    mq = np.ascontiguousarray(
        np.stack([np.concatenate([m, m], 1) for m in mas], 1))  # [128, 4, 1024]

    in_maps = []
    for c in range(NC):
        wq_cc = np.ascontiguousarray(wq[:, c * LH * D:(c + 1) * LH * D]).astype(BF16)
        wkv_cc = np.concatenate(
            [wk[:, c * D:(c + 1) * D], wv[:, c * D:(c + 1) * D]], 1).astype(BF16)
        in_maps.append({
            "xT": xT, "wq_c": wq_cc, "wkv_c": wkv_cc, "wo": wo_b,
            "ctq": ctq2, "ctk": ctk2, "mq": mq,
        })
    return in_maps


def get_nc():
    if "nc" not in _CACHE:
        _CACHE["nc"] = _build()
    return _CACHE["nc"]


def run(in_maps, **kwargs):
    nc = get_nc()
    return run_bass_kernel_spmd(nc, in_maps, core_ids=list(range(NC)), **kwargs)


def kernel(x, cos, sin, wq, wk, wv, wo):
    in_maps = _prep_inputs(x, cos, sin, wq, wk, wv, wo)
    res = run(in_maps)
    out = np.empty((T, HID), F32)
    for c in range(NC):
        out[TL * c:TL * (c + 1)] = res.results[c]["out"]
    return out.reshape(B, S, HID)


# revision 11
# speedup vs baseline: 1.1305x; 1.1305x over previous
"""Distributed Bass kernel for nn_Attention (B=2, S=2048, HID=2048, H=32, KVH=8, D=64).

Sharding (8 NeuronCores, uniform SPMD graph):
  - Head-parallel phase: core c owns kv-head c and its 4 GQA query heads.
    x replicated as xT [HID, B*S] bf16; per-core Q^T [256, 4096] (+RoPE,
    1/sqrt(D) folded into trig tables), K^T [64, 4096] (+RoPE, duplicated to
    rows 64:128), V [keys, 64|1] blocks with a ones-column for the softmax
    denominator.
  - Attention in S^T layout [keys, queries], processed per head-PAIR: the two
    heads of a pair sit on PE row-groups 0/64, so their K=64 S^T matmuls run
    concurrently (row-group tiling).  exp is split between the Scalar engine
    (true Exp) and the Vector engine (one-instruction Schraudolph bf16
    bit-trick exp ~ bitcast(int16(x*128/ln2 + 16248.75))) so neither engine
    bottlenecks.  Causal masking: rectangular blocks need no mask; the 4
    diagonal-band blocks per chunk use width-trimmed matmuls plus a 0/1 mask
    multiply (split Vector/GpSimd).
  - Per-unit drains DMA straight into the AllToAll staging buffer
    [NC, 130, TL] whose rows 64/129 carry the (bf16) softmax denominators —
    one collective per head-pair, no separate denominator collective.  Pair 0's
    collective overlaps pair 1's attention.
  - Token-parallel output projection: ao tiles normalized post-collective,
    out rows = ao.T @ wo per 512-col block; wo prefetched during attention.
"""

import os
import numpy as np
import ml_dtypes

import concourse.bass as bass
import concourse.mybir as mybir
import concourse.tile as tile
from concourse import bacc
from concourse.bass_utils import run_bass_kernel_spmd

BF16 = ml_dtypes.bfloat16
F32 = np.float32

B, S, HID = 2, 2048, 2048
H, KVH, D = 32, 8, 64
NC = 8                 # cores
T = B * S              # 4096 flat tokens
TL = T // NC           # 512 tokens per core (phase-2 output rows)
LH = H // NC           # 4 local q-heads per core
KB = 128               # key block
TC = 512               # phase-1 token streaming chunk / query chunk
NTC = T // TC          # 8 token chunks

SCHR_A = float(128.0 / np.log(2.0))
SCHR_B = float(127.0 * 128.0 - 7.25)

_CACHE = {}


def _build():
    fp32 = mybir.dt.float32
    bf16 = mybir.dt.bfloat16
    i16 = mybir.dt.int16

    # knobs (read at build time; defaults are the tuned config)
    DVE_EXP = float(os.environ.get("KDVE", "0.5"))   # fraction of exps on DVE
    DIAG_TRIM = os.environ.get("KDIAG", "1") == "1"  # width-trim diagonal blocks

    nc = bacc.Bacc("TRN2", target_bir_lowering=False, debug=False, num_devices=NC)

    xT = nc.dram_tensor("xT", [HID, T], bf16, kind="ExternalInput")
    wq_c = nc.dram_tensor("wq_c", [HID, LH * D], bf16, kind="ExternalInput")
    wkv_c = nc.dram_tensor("wkv_c", [HID, 2 * D], bf16, kind="ExternalInput")
    wo = nc.dram_tensor("wo", [HID, HID], bf16, kind="ExternalInput")
    ctq_d = nc.dram_tensor("ctq", [128, 2, T], fp32, kind="ExternalInput")
    ctk_d = nc.dram_tensor("ctk", [64, 2, T], fp32, kind="ExternalInput")
    mq_d = nc.dram_tensor("mq", [128, 2, 128], bf16, kind="ExternalInput")
    out_d = nc.dram_tensor("out", [TL, HID], fp32, kind="ExternalOutput")

    with tile.TileContext(nc) as tc:
        with (
            tc.tile_pool(name="persist", bufs=1) as persist,
            tc.tile_pool(name="stream", bufs=2) as stream,
            tc.tile_pool(name="trig", bufs=2) as trig,
            tc.tile_pool(name="work", bufs=2) as work,
            tc.tile_pool(name="psum", bufs=1, space="PSUM") as psum,
            tc.tile_pool(name="dram", bufs=1, space="DRAM") as dram,
        ):
            # ---- persistent tiles ----
            qT = [persist.tile([128, T], bf16, tag=f"qT{t}", name=f"qT{t}")
                  for t in range(2)]
            k2 = persist.tile([128, T], bf16, tag="k2", name="k2")
            vatt = [persist.tile([128, D + 1], bf16, tag=f"vatt{i}", name=f"vatt{i}")
                    for i in range(T // KB)]
            msq = persist.tile([128, 2, 128], bf16, tag="msq", name="msq")
            ident = persist.tile([128, 128], bf16, tag="ident", name="ident")

            # ---- weights (merged single-DMA loads) ----
            wq_sb = persist.tile([128, 16, LH * D], bf16, tag="wq", name="wq")
            wkv_sb = persist.tile([128, 16, 2 * D], bf16, tag="wkv", name="wkv")
            nc.sync.dma_start(wq_sb[:], wq_c.rearrange("(k p) m -> p k m", p=128))
            nc.scalar.dma_start(wkv_sb[:], wkv_c.rearrange("(k p) m -> p k m", p=128))

            # gpsimd queue: masks + ones-columns + identity (not startup-critical)
            nc.gpsimd.dma_start(msq[:], mq_d[:])
            for i in range(T // KB):
                nc.gpsimd.memset(vatt[i][:, D:D + 1], 1.0)
            from concourse.masks import make_identity
            make_identity(nc, ident[:])

            xT_r = xT.rearrange("(k p) t -> p k t", p=128)

            # ================= Phase 1: QKV projections + RoPE =================
            def rope(out_ap, ps, ct, st, npart):
                """out = ps*ct + swap32(ps)*st  (st carries the rotate-half sign)."""
                t1 = work.tile([128, TC], fp32, tag="rope_t1", name="t1")
                t2 = work.tile([128, TC], fp32, tag="rope_t2", name="t2")
                nc.vector.tensor_mul(t1[:npart, :], ps[:npart, :], ct[:npart, :])
                for base in range(0, npart, 64):
                    a, b2 = base, base + 32
                    nc.vector.tensor_mul(t2[a:a + 32, :], ps[b2:b2 + 32, :], st[a:a + 32, :])
                    nc.vector.tensor_mul(t2[b2:b2 + 32, :], ps[a:a + 32, :], st[b2:b2 + 32, :])
                nc.vector.tensor_add(out_ap, t1[:npart, :], t2[:npart, :])

            for tc8 in range(NTC):
                tsl = slice(TC * tc8, TC * (tc8 + 1))
                xt = stream.tile([128, 16, TC], bf16, tag="s", name=f"x{tc8}")
                nc.sync.dma_start(xt[:], xT_r[:, :, tsl])
                ctq = trig.tile([128, 2, TC], fp32, tag="ctq", name="ctq")
                ctk = trig.tile([64, 2, TC], fp32, tag="ctk", name="ctk")
                nc.scalar.dma_start(ctq[:], ctq_d[:, :, tsl])
                nc.scalar.dma_start(ctk[:], ctk_d[:, :, tsl])

                # Q^T: two 128-row tiles (2 heads each)
                for qt in range(2):
                    ps = psum.tile([128, 2 * TC], fp32, tag="big", bufs=3,
                                   name="ps_q")[:, 0:TC]
                    for k in range(16):
                        nc.tensor.matmul(ps[:], wq_sb[:, k, 128 * qt:128 * (qt + 1)],
                                         xt[:, k, :], start=(k == 0), stop=(k == 15))
                    rope(qT[qt][:, tsl], ps, ctq[:, 0, :], ctq[:, 1, :], 128)

                # K^T (rows 0:64) and V^T (rows 64:128) in one packed projection
                ps = psum.tile([128, 2 * TC], fp32, tag="big", bufs=3,
                               name="ps_kv")[:, 0:TC]
                for k in range(16):
                    nc.tensor.matmul(ps[:], wkv_sb[:, k, :], xt[:, k, :],
                                     start=(k == 0), stop=(k == 15))
                rope(k2[0:64, tsl], ps, ctk[:, 0, :], ctk[:, 1, :], 64)
                nc.scalar.copy(k2[64:128, tsl], k2[0:64, tsl])

                vt = work.tile([64, TC], bf16, tag="vt", name="vt")
                nc.scalar.copy(vt[:], ps[64:128, :])
                for j in range(TC // KB):
                    kbi = (TC // KB) * tc8 + j
                    pst = psum.tile([128, TC], bf16, tag="mm", bufs=2, name="ps_tr")
                    nc.tensor.transpose(pst[:, 0:64], vt[:, 128 * j:128 * (j + 1)],
                                        ident[0:64, 0:64])
                    nc.vector.tensor_copy(vatt[kbi][:, 0:D], pst[:, 0:64])

            # ---- wo prefetch: runs on DMA queues during attention ----
            # nt 0/1 get persistent tiles; nt 2/3 reuse the freed x-stream
            # slots (same [128, 16, 512] bf16 shape, x is done after phase 1).
            wo_r = wo.rearrange("(k p) n -> p k n", p=128)
            wo_sb = []
            w_eng = (nc.sync, nc.scalar, nc.gpsimd, nc.sync)
            for nt in range(4):
                if nt < 2:
                    t = persist.tile([128, 16, 512], bf16, tag=f"wo{nt}",
                                     name=f"wo{nt}")
                else:
                    t = stream.tile([128, 16, 512], bf16, tag="s", name=f"wo{nt}")
                w_eng[nt].dma_start(t[:], wo_r[:, :, 512 * nt:512 * (nt + 1)])
                wo_sb.append(t)

            # ================= Attention (head-pair parallel, causal) ==========
            a2a_in = [dram.tile([NC, 130, TL], bf16, tag=f"a2a_in{t}",
                                name=f"a2a_in{t}") for t in range(2)]
            a2a_out = [dram.tile([NC, 130, TL], bf16, tag=f"a2a_out{t}",
                                 name=f"a2a_out{t}") for t in range(2)]
            rstage = [dram.tile([2 * NC, TL], fp32, tag=f"rstage{t}",
                                name=f"rstage{t}") for t in range(2)]
            ao = {}
            mask_ctr = 0
            exp_acc = 0.0

            def attn_pair(pair):
                nonlocal mask_ctr, exp_acc
                qtile = qT[pair]
                for b in range(B):
                    for cq in range(4):
                        nkb = 4 * (cq + 1)
                        qs = S * b + TC * cq
                        j = 4 * b + cq          # destination core for this unit
                        psOa = psum.tile([128, 512], fp32, tag="mm", bufs=2,
                                         name="psOa")[0:D + 1, :]
                        psOb = psum.tile([128, 512], fp32, tag="mm", bufs=2,
                                         name="psOb")[0:D + 1, :]
                        for kb in range(nkb):
                            dj = kb - (nkb - 4)  # >=0: diagonal-band index
                            qoff = 128 * dj if (DIAG_TRIM and dj > 0) else 0
                            kpos = S * b + KB * kb
                            psS = psum.tile([128, 2 * TC], fp32, tag="big",
                                            bufs=3, name="psS")
                            ex = work.tile([128, 2 * TC], bf16, tag="ex", bufs=6,
                                           name="ex")
                            for h in range(2):
                                nc.tensor.matmul(
                                    psS[:, TC * h + qoff:TC * (h + 1)],
                                    k2[64 * h:64 * (h + 1), kpos:kpos + KB],
                                    qtile[64 * h:64 * (h + 1),
                                          qs + qoff:qs + TC],
                                    start=True, stop=True)
                            ps2 = psS.rearrange("p (h c) -> p h c", h=2)
                            ex2 = ex.rearrange("p (h c) -> p h c", h=2)
                            exi = ex.bitcast(i16).rearrange("p (h c) -> p h c", h=2)
                            exp_acc += DVE_EXP
                            if exp_acc >= 1.0:
                                exp_acc -= 1.0
                                nc.vector.tensor_scalar(
                                    exi[:, :, qoff:TC], ps2[:, :, qoff:TC],
                                    SCHR_A, SCHR_B,
                                    mybir.AluOpType.mult, mybir.AluOpType.add)
                            else:
                                nc.scalar.activation(
                                    ex2[:, :, qoff:TC], ps2[:, :, qoff:TC],
                                    mybir.ActivationFunctionType.Exp)
                            if dj >= 0:
                                # only the 128x128 diagonal square needs masking
                                meng = nc.gpsimd if (mask_ctr % 2 == 0) else nc.vector
                                mask_ctr += 1
                                meng.tensor_mul(ex2[:, :, qoff:qoff + 128],
                                                ex2[:, :, qoff:qoff + 128],
                                                msq[:, :, :])
                            for h, psO in ((0, psOa), (1, psOb)):
                                nc.tensor.matmul(
                                    psO[:, qoff:TC],
                                    vatt[(S // KB) * b + kb][:],
                                    ex2[:, h, qoff:TC],
                                    start=(kb == 0), stop=(kb == nkb - 1))
                        # drain: [attn 64 rows | den row] straight to a2a staging
                        for h, psO in ((0, psOa), (1, psOb)):
                            bounce = work.tile([D + 1, 512], bf16, tag="bounce",
                                               bufs=4, name="bounce")
                            if h == 0:
                                nc.scalar.copy(bounce[:], psO[:])
                            else:
                                nc.vector.tensor_copy(bounce[:], psO[:])
                            nc.sync.dma_start(
                                a2a_in[pair][j, 65 * h:65 * (h + 1), :], bounce[:])

            def a2a_post(pair):
                # den rows live at rows 64/129 of each a2a slot
                den_all = work.tile([2 * NC, TL], bf16, tag="den_all", bufs=2,
                                    name="den_all")
                a2a_v = a2a_out[pair].rearrange("a (x b) c -> (a x) b c", x=2)
                nc.sync.dma_start(den_all[:], a2a_v[:, 64, :])
                rall = work.tile([2 * NC, TL], fp32, tag="rall", bufs=2,
                                 name="rall")
                nc.vector.reciprocal(rall[:], den_all[:])
                nc.sync.dma_start(rstage[pair][:], rall[:])
                for r in range(NC):
                    kk = 2 * r + pair
                    tl_ = persist.tile([128, TL], bf16, tag=f"ao{kk}",
                                       name=f"ao{kk}")
                    nc.sync.dma_start(tl_[0:64, :], a2a_out[pair][r, 0:64, :])
                    nc.sync.dma_start(tl_[64:128, :], a2a_out[pair][r, 65:129, :])
                    rb2 = work.tile([128, TL], fp32, tag="rb2", bufs=2,
                                    name="rb2")
                    beng = nc.gpsimd if (r % 2 == 0) else nc.sync
                    beng.dma_start(
                        rb2[0:64, :],
                        rstage[pair][2 * r:2 * r + 1, :].broadcast_to([64, TL]))
                    beng.dma_start(
                        rb2[64:128, :],
                        rstage[pair][2 * r + 1:2 * r + 2, :].broadcast_to([64, TL]))
                    nc.vector.tensor_mul(tl_[:], tl_[:], rb2[:])
                    ao[kk] = tl_

            def a2a_go(pair):
                nc.gpsimd.collective_compute(
                    "AllToAll", mybir.AluOpType.bypass,
                    replica_groups=[list(range(NC))],
                    ins=[a2a_in[pair].opt()], outs=[a2a_out[pair].opt()])

            # pair-0 post-collective work is emitted AFTER pair-1's attention
            # so the FIFO engine queues never make pair-1 wait on the a2a.
            attn_pair(0)
            a2a_go(0)
            attn_pair(1)
            a2a_post(0)
            a2a_go(1)
            a2a_post(1)

            # ================= Phase 2: output projection =====================
            # kk-even (pair-0) contraction first: those matmuls only need the
            # pair-0 ao tiles, so the PE fills part of the a2a-1 wait.
            kk_order = [2 * r for r in range(8)] + [2 * r + 1 for r in range(8)]
            pg = 0
            for nt in range(4):
                for tt in range(TL // 128):
                    if pg % 5 < 3:
                        ps = psum.tile([128, 2 * TC], fp32, tag="big", bufs=3,
                                       name="ps_o")[:, 0:512]
                    else:
                        ps = psum.tile([128, 512], fp32, tag="mm", bufs=2,
                                       name="ps_o")
                    pg += 1
                    for ki, kk in enumerate(kk_order):
                        nc.tensor.matmul(ps[:], ao[kk][:, 128 * tt:128 * (tt + 1)],
                                         wo_sb[nt][:, kk, :],
                                         start=(ki == 0), stop=(ki == 15))
                    ob = work.tile([128, 512], fp32, tag="ob", bufs=2, name="ob")
                    if (nt + tt) % 2 == 0:
                        nc.scalar.copy(ob[:], ps[:])
                    else:
                        nc.vector.tensor_copy(ob[:], ps[:])
                    oeng = nc.gpsimd if (nt + tt) % 2 == 0 else nc.sync
                    oeng.dma_start(out_d[128 * tt:128 * (tt + 1),
                                         512 * nt:512 * (nt + 1)], ob[:])

    nc.compile()
    return nc


def _prep_inputs(x, cos, sin, wq, wk, wv, wo):
    x = np.asarray(x, F32)
    cos = np.asarray(cos, F32)
    sin = np.asarray(sin, F32)
    wq = np.asarray(wq, F32)
    wk = np.asarray(wk, F32)
    wv = np.asarray(wv, F32)
    wo = np.asarray(wo, F32)

    xT = np.ascontiguousarray(x.reshape(T, HID).T).astype(BF16)
    wo_b = wo.astype(BF16)

    pos = np.arange(T) % S
    sign = np.concatenate([-np.ones(D // 2, F32), np.ones(D // 2, F32)])
    ctk = np.ascontiguousarray(cos[pos].T)                      # [64, T]
    stk = np.ascontiguousarray((sin[pos] * sign).T)             # [64, T]
    ctk2 = np.ascontiguousarray(np.stack([ctk, stk], 1))        # [64, 2, T]
    scale = F32(1.0 / np.sqrt(D))
    ctq2 = np.ascontiguousarray(
        np.stack([np.concatenate([ctk, ctk], 0) * scale,
                  np.concatenate([stk, stk], 0) * scale], 1))   # [128, 2, T]

    # single 128x128 lower-tri mask, doubled for the two heads of a pair
    kl = np.arange(128)
    msq = (kl[None, :] >= kl[:, None]).astype(BF16)
    mq = np.ascontiguousarray(np.stack([msq, msq], 1))          # [128, 2, 128]

    in_maps = []
    for c in range(NC):
        wq_cc = np.ascontiguousarray(wq[:, c * LH * D:(c + 1) * LH * D]).astype(BF16)
        wkv_cc = np.concatenate(
            [wk[:, c * D:(c + 1) * D], wv[:, c * D:(c + 1) * D]], 1).astype(BF16)
        in_maps.append({
            "xT": xT, "wq_c": wq_cc, "wkv_c": wkv_cc, "wo": wo_b,
            "ctq": ctq2, "ctk": ctk2, "mq": mq,
        })
    return in_maps


def get_nc():
    if "nc" not in _CACHE:
        _CACHE["nc"] = _build()
    return _CACHE["nc"]


def run(in_maps, **kwargs):
    nc = get_nc()
    return run_bass_kernel_spmd(nc, in_maps, core_ids=list(range(NC)), **kwargs)


def kernel(x, cos, sin, wq, wk, wv, wo):
    in_maps = _prep_inputs(x, cos, sin, wq, wk, wv, wo)
    res = run(in_maps)
    out = np.empty((T, HID), F32)
    for c in range(NC):
        out[TL * c:TL * (c + 1)] = res.results[c]["out"]
    return out.reshape(B, S, HID)


# revision 25
# speedup vs baseline: 1.1849x; 1.0481x over previous
"""Distributed Bass kernel for nn_Attention (B=2, S=2048, HID=2048, H=32, KVH=8, D=64).

Sharding (8 NeuronCores, uniform SPMD graph):
  - Head-parallel phase: core c owns kv-head c and its 4 GQA query heads.
    x replicated as xT [HID, B*S] bf16; per-core Q^T [256, 4096] (+RoPE,
    1/sqrt(D) folded into trig tables), K^T [64, 4096] (+RoPE, duplicated to
    rows 64:128), V [keys, 64|1] blocks with a ones-column for the softmax
    denominator.
  - Attention in S^T layout [keys, queries], processed per head-PAIR: the two
    heads of a pair sit on PE row-groups 0/64, so their K=64 S^T matmuls run
    concurrently (row-group tiling).  exp is split between the Scalar engine
    (true Exp) and the Vector engine (one-instruction Schraudolph bf16
    bit-trick exp ~ bitcast(int16(x*128/ln2 + 16248.75))) so neither engine
    bottlenecks.  Causal masking: rectangular blocks need no mask; the 4
    diagonal-band blocks per chunk use width-trimmed matmuls plus a 0/1 mask
    multiply (split Vector/GpSimd).
  - Per-unit drains DMA straight into the AllToAll staging buffer
    [NC, 130, TL] whose rows 64/129 carry the (bf16) softmax denominators —
    one collective per head-pair, no separate denominator collective.  Pair 0's
    collective overlaps pair 1's attention.
  - Token-parallel output projection: ao tiles normalized post-collective,
    out rows = ao.T @ wo per 512-col block; wo prefetched during attention.
"""

import os
import numpy as np
import ml_dtypes

import concourse.bass as bass
import concourse.mybir as mybir
import concourse.tile as tile
from concourse import bacc
from concourse.bass_utils import run_bass_kernel_spmd

BF16 = ml_dtypes.bfloat16
F32 = np.float32

B, S, HID = 2, 2048, 2048
H, KVH, D = 32, 8, 64
NC = 8                 # cores
T = B * S              # 4096 flat tokens
TL = T // NC           # 512 tokens per core (phase-2 output rows)
LH = H // NC           # 4 local q-heads per core
KB = 128               # key block
TC = 512               # phase-1 token streaming chunk / query chunk
NTC = T // TC          # 8 token chunks

SCHR_A = float(128.0 / np.log(2.0))
SCHR_B = float(127.0 * 128.0 - 7.25)

_CACHE = {}


def _build():
    fp32 = mybir.dt.float32
    bf16 = mybir.dt.bfloat16
    i16 = mybir.dt.int16

    # knobs (read at build time; defaults are the tuned config)
    DVE_EXP = float(os.environ.get("KDVE", "0.5"))   # fraction of exps on DVE
    DIAG_TRIM = os.environ.get("KDIAG", "1") == "1"  # width-trim diagonal blocks

    nc = bacc.Bacc("TRN2", target_bir_lowering=False, debug=False, num_devices=NC)

    xT = nc.dram_tensor("xT", [HID, T], bf16, kind="ExternalInput")
    wq_c = nc.dram_tensor("wq_c", [HID, LH * D], bf16, kind="ExternalInput")
    wkv_c = nc.dram_tensor("wkv_c", [HID, 2 * D], bf16, kind="ExternalInput")
    wo = nc.dram_tensor("wo", [HID, HID], bf16, kind="ExternalInput")
    ctq_d = nc.dram_tensor("ctq", [128, 2, T], fp32, kind="ExternalInput")
    ctk_d = nc.dram_tensor("ctk", [64, 2, T], fp32, kind="ExternalInput")
    mq_d = nc.dram_tensor("mq", [128, 2, 128], bf16, kind="ExternalInput")
    out_d = nc.dram_tensor("out", [TL, HID], fp32, kind="ExternalOutput")

    with tile.TileContext(nc) as tc:
        with (
            tc.tile_pool(name="persist", bufs=1) as persist,
            tc.tile_pool(name="stream", bufs=2) as stream,
            tc.tile_pool(name="trig", bufs=2) as trig,
            tc.tile_pool(name="work", bufs=2) as work,
            tc.tile_pool(name="psum", bufs=1, space="PSUM") as psum,
            tc.tile_pool(name="dram", bufs=1, space="DRAM") as dram,
        ):
            # ---- persistent tiles ----
            qT = [persist.tile([128, T], bf16, tag=f"qT{t}", name=f"qT{t}")
                  for t in range(2)]
            k2 = persist.tile([128, T], bf16, tag="k2", name="k2")
            vatt = [persist.tile([128, D + 1], bf16, tag=f"vatt{i}", name=f"vatt{i}")
                    for i in range(T // KB)]
            msq = persist.tile([128, 2, 128], bf16, tag="msq", name="msq")
            ident = persist.tile([128, 128], bf16, tag="ident", name="ident")

            # ---- weights (merged single-DMA loads) ----
            wq_sb = persist.tile([128, 16, LH * D], bf16, tag="wq", name="wq")
            wkv_sb = persist.tile([128, 16, 2 * D], bf16, tag="wkv", name="wkv")
            wq_r = wq_c.rearrange("(k p) m -> p k m", p=128)
            for g in range(4):
                nc.scalar.dma_start(wq_sb[:, 4 * g:4 * (g + 1), :],
                                    wq_r[:, 4 * g:4 * (g + 1), :])
            nc.scalar.dma_start(wkv_sb[:], wkv_c.rearrange("(k p) m -> p k m", p=128))

            from concourse.masks import make_identity
            make_identity(nc, ident[:])

            xT_r = xT.rearrange("(k p) t -> p k t", p=128)

            # ================= Phase 1: QKV projections + RoPE =================
            def rope(out_ap, ps, ct, st, npart):
                """out = ps*ct + swap32(ps)*st  (st carries the rotate-half sign)."""
                t1 = work.tile([128, TC], fp32, tag="rope_t1", name="t1")
                t2 = work.tile([128, TC], fp32, tag="rope_t2", name="t2")
                nc.vector.tensor_mul(t1[:npart, :], ps[:npart, :], ct[:npart, :])
                for base in range(0, npart, 64):
                    a, b2 = base, base + 32
                    nc.vector.tensor_mul(t2[a:a + 32, :], ps[b2:b2 + 32, :], st[a:a + 32, :])
                    nc.vector.tensor_mul(t2[b2:b2 + 32, :], ps[a:a + 32, :], st[b2:b2 + 32, :])
                nc.vector.tensor_add(out_ap, t1[:npart, :], t2[:npart, :])

            for tc8 in range(NTC):
                tsl = slice(TC * tc8, TC * (tc8 + 1))
                xt = stream.tile([128, 16, TC], bf16, tag="s", name=f"x{tc8}")
                if tc8 == 0:
                    # split the first chunk so the first matmuls start early
                    for g, eng in enumerate((nc.sync, nc.gpsimd, nc.sync,
                                             nc.gpsimd)):
                        eng.dma_start(xt[:, 4 * g:4 * (g + 1), :],
                                      xT_r[:, 4 * g:4 * (g + 1), tsl])
                else:
                    nc.sync.dma_start(xt[:], xT_r[:, :, tsl])
                ctq = trig.tile([128, 2, TC], fp32, tag="ctq", name="ctq")
                ctk = trig.tile([64, 2, TC], fp32, tag="ctk", name="ctk")
                nc.scalar.dma_start(ctq[:], ctq_d[:, :, tsl])
                nc.scalar.dma_start(ctk[:], ctk_d[:, :, tsl])

                # Q^T: two 128-row tiles (2 heads each)
                for qt in range(2):
                    ps = psum.tile([128, 2 * TC], fp32, tag="big", bufs=3,
                                   name="ps_q")[:, 0:TC]
                    for k in range(16):
                        nc.tensor.matmul(ps[:], wq_sb[:, k, 128 * qt:128 * (qt + 1)],
                                         xt[:, k, :], start=(k == 0), stop=(k == 15))
                    rope(qT[qt][:, tsl], ps, ctq[:, 0, :], ctq[:, 1, :], 128)

                # K^T (rows 0:64) and V^T (rows 64:128) in one packed projection
                ps = psum.tile([128, 2 * TC], fp32, tag="big", bufs=3,
                               name="ps_kv")[:, 0:TC]
                for k in range(16):
                    nc.tensor.matmul(ps[:], wkv_sb[:, k, :], xt[:, k, :],
                                     start=(k == 0), stop=(k == 15))
                rope(k2[0:64, tsl], ps, ctk[:, 0, :], ctk[:, 1, :], 64)
                nc.scalar.copy(k2[64:128, tsl], k2[0:64, tsl])

                vt = work.tile([64, TC], bf16, tag="vt", name="vt")
                nc.scalar.copy(vt[:], ps[64:128, :])
                for j in range(TC // KB):
                    kbi = (TC // KB) * tc8 + j
                    pst = psum.tile([128, TC], bf16, tag="mm", bufs=2, name="ps_tr")
                    nc.tensor.transpose(pst[:, 0:64], vt[:, 128 * j:128 * (j + 1)],
                                        ident[0:64, 0:64])
                    nc.vector.tensor_copy(vatt[kbi][:, 0:D], pst[:, 0:64])

            # mask + ones-columns: only read from attention on; emitting them
            # here keeps the startup DMA queues clear for x/weights.
            nc.gpsimd.dma_start(msq[:], mq_d[:])
            for i in range(T // KB):
                nc.gpsimd.memset(vatt[i][:, D:D + 1], 1.0)

            # ---- wo prefetch: runs on DMA queues during attention ----
            # nt 0/1 get persistent tiles; nt 2/3 reuse the freed x-stream
            # slots (same [128, 16, 512] bf16 shape, x is done after phase 1).
            wo_r = wo.rearrange("(k p) n -> p k n", p=128)
            wo_sb = []
            w_eng = (nc.sync, nc.scalar, nc.gpsimd, nc.sync)
            for nt in range(4):
                if nt < 2:
                    t = persist.tile([128, 16, 512], bf16, tag=f"wo{nt}",
                                     name=f"wo{nt}")
                else:
                    t = stream.tile([128, 16, 512], bf16, tag="s", name=f"wo{nt}")
                w_eng[nt].dma_start(t[:], wo_r[:, :, 512 * nt:512 * (nt + 1)])
                wo_sb.append(t)

            # ================= Attention (head-pair parallel, causal) ==========
            a2a_in = [dram.tile([NC, 130, TL], bf16, tag=f"a2a_in{t}",
                                name=f"a2a_in{t}") for t in range(2)]
            a2a_out = [dram.tile([NC, 130, TL], bf16, tag=f"a2a_out{t}",
                                 name=f"a2a_out{t}") for t in range(2)]
            den_sb = [persist.tile([2 * NC, TC], bf16, tag=f"den{t}",
                                   name=f"den{t}") for t in range(2)]
            ao = {}
            mask_ctr = 0
            exp_acc = 0.0

            def attn_pair(pair):
                nonlocal mask_ctr, exp_acc
                qtile = qT[pair]
                for b in range(B):
                    for cq in range(4):
                        nkb = 4 * (cq + 1)
                        qs = S * b + TC * cq
                        j = 4 * b + cq          # destination core for this unit
                        psOa = psum.tile([128, 512], fp32, tag="mm", bufs=2,
                                         name="psOa")[0:D + 1, :]
                        psOb = psum.tile([128, 512], fp32, tag="mm", bufs=2,
                                         name="psOb")[0:D + 1, :]
                        for kb in range(nkb):
                            dj = kb - (nkb - 4)  # >=0: diagonal-band index
                            qoff = 128 * dj if (DIAG_TRIM and dj > 0) else 0
                            kpos = S * b + KB * kb
                            psS = psum.tile([128, 2 * TC], fp32, tag="big",
                                            bufs=3, name="psS")
                            ex = work.tile([128, 2 * TC], bf16, tag="ex", bufs=6,
                                           name="ex")
                            for h in range(2):
                                nc.tensor.matmul(
                                    psS[:, TC * h + qoff:TC * (h + 1)],
                                    k2[64 * h:64 * (h + 1), kpos:kpos + KB],
                                    qtile[64 * h:64 * (h + 1),
                                          qs + qoff:qs + TC],
                                    start=True, stop=True)
                            ps2 = psS.rearrange("p (h c) -> p h c", h=2)
                            ex2 = ex.rearrange("p (h c) -> p h c", h=2)
                            exi = ex.bitcast(i16).rearrange("p (h c) -> p h c", h=2)
                            exp_acc += DVE_EXP
                            if exp_acc >= 1.0:
                                exp_acc -= 1.0
                                nc.vector.tensor_scalar(
                                    exi[:, :, qoff:TC], ps2[:, :, qoff:TC],
                                    SCHR_A, SCHR_B,
                                    mybir.AluOpType.mult, mybir.AluOpType.add)
                            else:
                                nc.scalar.activation(
                                    ex2[:, :, qoff:TC], ps2[:, :, qoff:TC],
                                    mybir.ActivationFunctionType.Exp)
                            if dj >= 0:
                                # only the 128x128 diagonal square needs masking
                                meng = nc.gpsimd if (mask_ctr % 2 == 0) else nc.vector
                                mask_ctr += 1
                                meng.tensor_mul(ex2[:, :, qoff:qoff + 128],
                                                ex2[:, :, qoff:qoff + 128],
                                                msq[:, :, :])
                            for h, psO in ((0, psOa), (1, psOb)):
                                nc.tensor.matmul(
                                    psO[:, qoff:TC],
                                    vatt[(S // KB) * b + kb][:],
                                    ex2[:, h, qoff:TC],
                                    start=(kb == 0), stop=(kb == nkb - 1))
                        # drain: attn rows straight to a2a staging; den row to
                        # the local den staging tile (reciprocal'd pre-a2a)
                        for h, psO in ((0, psOa), (1, psOb)):
                            bounce = work.tile([D + 1, 512], bf16, tag="bounce",
                                               bufs=4, name="bounce")
                            if h == 0:
                                nc.scalar.copy(bounce[:], psO[:])
                            else:
                                nc.vector.tensor_copy(bounce[:], psO[:])
                            nc.sync.dma_start(
                                a2a_in[pair][j, 65 * h:65 * h + D, :],
                                bounce[0:D, :])
                            nc.sync.dma_start(
                                den_sb[pair][2 * j + h:2 * j + h + 1, :],
                                bounce[D:D + 1, :])

            i32 = mybir.dt.int32
            RMAGIC = 0x7EF127EA

            def recip_stage(pair):
                # 1/den on GpSimd (float bit-trick + 2 Newton steps) so neither
                # the Scalar nor Vector queue is touched; the reciprocals ride
                # the a2a in the payload's den rows (64/129 of each slot).
                ds = work.tile([2 * NC, TC], fp32, tag="rc_ds", bufs=2, name="ds")
                nc.gpsimd.tensor_copy(ds[:], den_sb[pair][:])      # bf16 -> f32
                bf = work.tile([2 * NC, TC], fp32, tag="rc_bf", bufs=2, name="bf")
                nc.gpsimd.tensor_copy(bf[:], ds.bitcast(i32)[:])   # bits as f32
                nc.gpsimd.tensor_scalar(bf[:], bf[:], -1.0, float(RMAGIC),
                                        mybir.AluOpType.mult,
                                        mybir.AluOpType.add)
                r0i = work.tile([2 * NC, TC], i32, tag="r0i", bufs=2, name="r0i")
                nc.gpsimd.tensor_copy(r0i[:], bf[:])               # back to bits
                r0 = r0i.bitcast(fp32)
                t = work.tile([2 * NC, TC], fp32, tag="rc_t", bufs=2, name="rc_t")
                for _ in range(2):
                    nc.gpsimd.tensor_mul(t[:], ds[:], r0[:])
                    nc.gpsimd.tensor_scalar(t[:], t[:], -1.0, 2.0,
                                            mybir.AluOpType.mult,
                                            mybir.AluOpType.add)
                    nc.gpsimd.tensor_mul(r0[:], r0[:], t[:])
                rb = work.tile([2 * NC, TC], bf16, tag="rcb", bufs=2, name="rcb")
                nc.gpsimd.tensor_copy(rb[:], r0[:])
                nc.gpsimd.dma_start(
                    a2a_in[pair].rearrange("a (x b) c -> (a x) b c", x=2)[:, 64, :],
                    rb[:])

            def a2a_post(pair):
                # payload den rows already hold 1/den (bf16): just broadcast
                # straight from the a2a output in DRAM and multiply.
                a2a_v = a2a_out[pair].rearrange("a (x b) c -> (a x) b c", x=2)
                for r in range(NC):
                    kk = 2 * r + pair
                    tl_ = persist.tile([128, TL], bf16, tag=f"ao{kk}",
                                       name=f"ao{kk}")
                    nc.sync.dma_start(tl_[0:64, :], a2a_out[pair][r, 0:64, :])
                    nc.sync.dma_start(tl_[64:128, :], a2a_out[pair][r, 65:129, :])
                    # tag "bounce": ring reuse gives the scheduler a visible
                    # dependency on late pair-1 work, keeping these off the
                    # queues until the a2a is genuinely near done.
                    rb2 = work.tile([128, TL], bf16, tag="bounce", bufs=4,
                                    name="rb2")
                    beng = nc.gpsimd if (r % 2 == 0) else nc.sync
                    beng.dma_start(
                        rb2[0:64, :],
                        a2a_v[2 * r:2 * r + 1, 64, :].broadcast_to([64, TL]))
                    beng.dma_start(
                        rb2[64:128, :],
                        a2a_v[2 * r + 1:2 * r + 2, 64, :].broadcast_to([64, TL]))
                    nc.vector.tensor_mul(tl_[:], tl_[:], rb2[:])
                    ao[kk] = tl_

            def a2a_go(pair):
                nc.gpsimd.collective_compute(
                    "AllToAll", mybir.AluOpType.bypass,
                    replica_groups=[list(range(NC))],
                    ins=[a2a_in[pair].opt()], outs=[a2a_out[pair].opt()])

            # pair-0 post-collective work is emitted AFTER pair-1's attention
            # (and demoted in scheduler priority) so the FIFO engine queues
            # never make pair-1's attention wait on the a2a.
            attn_pair(0)
            recip_stage(0)
            a2a_go(0)
            attn_pair(1)
            recip_stage(1)
            a2a_go(1)
            with tc.high_priority(offset=-500000):
                a2a_post(0)
            a2a_post(1)

            # ================= Phase 2: output projection =====================
            # kk-even (pair-0) contraction first: those matmuls only need the
            # pair-0 ao tiles, so the PE fills part of the a2a-1 wait.
            kk_order = [2 * r for r in range(8)] + [2 * r + 1 for r in range(8)]
            pg = 0
            for nt in range(4):
                for tt in range(TL // 128):
                    if pg % 5 < 3:
                        ps = psum.tile([128, 2 * TC], fp32, tag="big", bufs=3,
                                       name="ps_o")[:, 0:512]
                    else:
                        ps = psum.tile([128, 512], fp32, tag="mm", bufs=2,
                                       name="ps_o")
                    pg += 1
                    for ki, kk in enumerate(kk_order):
                        nc.tensor.matmul(ps[:], ao[kk][:, 128 * tt:128 * (tt + 1)],
                                         wo_sb[nt][:, kk, :],
                                         start=(ki == 0), stop=(ki == 15))
                    ob = work.tile([128, 512], fp32, tag="ob", bufs=2, name="ob")
                    if (nt + tt) % 2 == 0:
                        nc.scalar.copy(ob[:], ps[:])
                    else:
                        nc.vector.tensor_copy(ob[:], ps[:])
                    oeng = nc.gpsimd if (nt + tt) % 2 == 0 else nc.sync
                    oeng.dma_start(out_d[128 * tt:128 * (tt + 1),
                                         512 * nt:512 * (nt + 1)], ob[:])

    nc.compile()
    return nc


def _prep_inputs(x, cos, sin, wq, wk, wv, wo):
    x = np.asarray(x, F32)
    cos = np.asarray(cos, F32)
    sin = np.asarray(sin, F32)
    wq = np.asarray(wq, F32)
    wk = np.asarray(wk, F32)
    wv = np.asarray(wv, F32)
    wo = np.asarray(wo, F32)

    xT = np.ascontiguousarray(x.reshape(T, HID).T).astype(BF16)
    wo_b = wo.astype(BF16)

    pos = np.arange(T) % S
    sign = np.concatenate([-np.ones(D // 2, F32), np.ones(D // 2, F32)])
    ctk = np.ascontiguousarray(cos[pos].T)                      # [64, T]
    stk = np.ascontiguousarray((sin[pos] * sign).T)             # [64, T]
    ctk2 = np.ascontiguousarray(np.stack([ctk, stk], 1))        # [64, 2, T]
    scale = F32(1.0 / np.sqrt(D))
    ctq2 = np.ascontiguousarray(
        np.stack([np.concatenate([ctk, ctk], 0) * scale,
                  np.concatenate([stk, stk], 0) * scale], 1))   # [128, 2, T]

    # single 128x128 lower-tri mask, doubled for the two heads of a pair
    kl = np.arange(128)
    msq = (kl[None, :] >= kl[:, None]).astype(BF16)
    mq = np.ascontiguousarray(np.stack([msq, msq], 1))          # [128, 2, 128]

    in_maps = []
    for c in range(NC):
        wq_cc = np.ascontiguousarray(wq[:, c * LH * D:(c + 1) * LH * D]).astype(BF16)
        wkv_cc = np.concatenate(
            [wk[:, c * D:(c + 1) * D], wv[:, c * D:(c + 1) * D]], 1).astype(BF16)
        in_maps.append({
            "xT": xT, "wq_c": wq_cc, "wkv_c": wkv_cc, "wo": wo_b,
            "ctq": ctq2, "ctk": ctk2, "mq": mq,
        })
    return in_maps


def get_nc():
    if "nc" not in _CACHE:
        _CACHE["nc"] = _build()
    return _CACHE["nc"]


def run(in_maps, **kwargs):
    nc = get_nc()
    return run_bass_kernel_spmd(nc, in_maps, core_ids=list(range(NC)), **kwargs)


def kernel(x, cos, sin, wq, wk, wv, wo):
    in_maps = _prep_inputs(x, cos, sin, wq, wk, wv, wo)
    res = run(in_maps)
    out = np.empty((T, HID), F32)
    for c in range(NC):
        out[TL * c:TL * (c + 1)] = res.results[c]["out"]
    return out.reshape(B, S, HID)


# revision 28
# speedup vs baseline: 1.2121x; 1.0229x over previous
"""Distributed Bass kernel for nn_Attention (B=2, S=2048, HID=2048, H=32, KVH=8, D=64).

Sharding (8 NeuronCores, uniform SPMD graph):
  - Head-parallel phase: core c owns kv-head c and its 4 GQA query heads.
    x replicated as xT [HID, B*S] bf16; per-core Q^T [256, 4096] (+RoPE,
    1/sqrt(D) folded into trig tables), K^T [64, 4096] (+RoPE, duplicated to
    rows 64:128), V [keys, 64|1] blocks with a ones-column for the softmax
    denominator.
  - Attention in S^T layout [keys, queries], processed per head-PAIR: the two
    heads of a pair sit on PE row-groups 0/64, so their K=64 S^T matmuls run
    concurrently (row-group tiling).  exp is split between the Scalar engine
    (true Exp) and the Vector engine (one-instruction Schraudolph bf16
    bit-trick exp ~ bitcast(int16(x*128/ln2 + 16248.75))) so neither engine
    bottlenecks.  Causal masking: rectangular blocks need no mask; the 4
    diagonal-band blocks per chunk use width-trimmed matmuls plus a 0/1 mask
    multiply (split Vector/GpSimd).
  - Per-unit drains DMA straight into the AllToAll staging buffer
    [NC, 130, TL] whose rows 64/129 carry the (bf16) softmax denominators —
    one collective per head-pair, no separate denominator collective.  Pair 0's
    collective overlaps pair 1's attention.
  - Token-parallel output projection: ao tiles normalized post-collective,
    out rows = ao.T @ wo per 512-col block; wo prefetched during attention.
"""

import os
import numpy as np
import ml_dtypes

import concourse.bass as bass
import concourse.mybir as mybir
import concourse.tile as tile
from concourse import bacc
from concourse.bass_utils import run_bass_kernel_spmd

BF16 = ml_dtypes.bfloat16
F32 = np.float32

B, S, HID = 2, 2048, 2048
H, KVH, D = 32, 8, 64
NC = 8                 # cores
T = B * S              # 4096 flat tokens
TL = T // NC           # 512 tokens per core (phase-2 output rows)
LH = H // NC           # 4 local q-heads per core
KB = 128               # key block
TC = 512               # phase-1 token streaming chunk / query chunk
NTC = T // TC          # 8 token chunks

SCHR_A = float(128.0 / np.log(2.0))
SCHR_B = float(127.0 * 128.0 - 7.25)

_CACHE = {}


def _build():
    fp32 = mybir.dt.float32
    bf16 = mybir.dt.bfloat16
    i16 = mybir.dt.int16

    # knobs (read at build time; defaults are the tuned config)
    DVE_EXP = float(os.environ.get("KDVE", "0.5"))   # fraction of exps on DVE
    DIAG_TRIM = os.environ.get("KDIAG", "1") == "1"  # width-trim diagonal blocks

    nc = bacc.Bacc("TRN2", target_bir_lowering=False, debug=False, num_devices=NC)

    xT = nc.dram_tensor("xT", [HID, T], bf16, kind="ExternalInput")
    wq_c = nc.dram_tensor("wq_c", [HID, LH * D], bf16, kind="ExternalInput")
    wkv_c = nc.dram_tensor("wkv_c", [HID, 2 * D], bf16, kind="ExternalInput")
    wo = nc.dram_tensor("wo", [HID, HID], bf16, kind="ExternalInput")
    ctq_d = nc.dram_tensor("ctq", [128, 2, T], fp32, kind="ExternalInput")
    ctk_d = nc.dram_tensor("ctk", [64, 2, T], fp32, kind="ExternalInput")
    mq_d = nc.dram_tensor("mq", [128, 2, 128], bf16, kind="ExternalInput")
    out_d = nc.dram_tensor("out", [TL, HID], fp32, kind="ExternalOutput")

    with tile.TileContext(nc) as tc:
        with (
            tc.tile_pool(name="persist", bufs=1) as persist,
            tc.tile_pool(name="stream", bufs=2) as stream,
            tc.tile_pool(name="trig", bufs=2) as trig,
            tc.tile_pool(name="work", bufs=2) as work,
            tc.tile_pool(name="psum", bufs=1, space="PSUM") as psum,
            tc.tile_pool(name="dram", bufs=1, space="DRAM") as dram,
        ):
            # ---- persistent tiles ----
            qT = [persist.tile([128, T], bf16, tag=f"qT{t}", name=f"qT{t}")
                  for t in range(2)]
            k2 = persist.tile([128, T], bf16, tag="k2", name="k2")
            vatt = [persist.tile([128, D + 1], bf16, tag=f"vatt{i}", name=f"vatt{i}")
                    for i in range(T // KB)]
            msq = persist.tile([128, 2, 128], bf16, tag="msq", name="msq")
            ident = persist.tile([128, 128], bf16, tag="ident", name="ident")

            # ---- weights (merged single-DMA loads) ----
            wq_sb = persist.tile([128, 16, LH * D], bf16, tag="wq", name="wq")
            wkv_sb = persist.tile([128, 16, 2 * D], bf16, tag="wkv", name="wkv")
            wq_r = wq_c.rearrange("(k p) m -> p k m", p=128)
            for g in range(4):
                nc.scalar.dma_start(wq_sb[:, 4 * g:4 * (g + 1), :],
                                    wq_r[:, 4 * g:4 * (g + 1), :])
            nc.scalar.dma_start(wkv_sb[:], wkv_c.rearrange("(k p) m -> p k m", p=128))

            from concourse.masks import make_identity
            make_identity(nc, ident[:])

            xT_r = xT.rearrange("(k p) t -> p k t", p=128)

            # ================= Phase 1: QKV projections + RoPE =================
            def rope(out_ap, ps, ct, st, npart):
                """out = ps*ct + swap32(ps)*st  (st carries the rotate-half sign)."""
                t1 = work.tile([128, TC], fp32, tag="rope_t1", name="t1")
                t2 = work.tile([128, TC], fp32, tag="rope_t2", name="t2")
                nc.vector.tensor_mul(t1[:npart, :], ps[:npart, :], ct[:npart, :])
                for base in range(0, npart, 64):
                    a, b2 = base, base + 32
                    nc.vector.tensor_mul(t2[a:a + 32, :], ps[b2:b2 + 32, :], st[a:a + 32, :])
                    nc.vector.tensor_mul(t2[b2:b2 + 32, :], ps[a:a + 32, :], st[b2:b2 + 32, :])
                nc.vector.tensor_add(out_ap, t1[:npart, :], t2[:npart, :])

            for tc8 in range(NTC):
                tsl = slice(TC * tc8, TC * (tc8 + 1))
                xt = stream.tile([128, 16, TC], bf16, tag="s", name=f"x{tc8}")
                if tc8 == 0:
                    # split the first chunk so the first matmuls start early
                    for g, eng in enumerate((nc.sync, nc.gpsimd, nc.sync,
                                             nc.gpsimd)):
                        eng.dma_start(xt[:, 4 * g:4 * (g + 1), :],
                                      xT_r[:, 4 * g:4 * (g + 1), tsl])
                else:
                    nc.sync.dma_start(xt[:, 0:8, :], xT_r[:, 0:8, tsl])
                    nc.scalar.dma_start(xt[:, 8:16, :], xT_r[:, 8:16, tsl])
                ctq = trig.tile([128, 2, TC], fp32, tag="ctq", name="ctq")
                ctk = trig.tile([64, 2, TC], fp32, tag="ctk", name="ctk")
                nc.scalar.dma_start(ctq[:], ctq_d[:, :, tsl])
                nc.scalar.dma_start(ctk[:], ctk_d[:, :, tsl])

                # Q^T: two 128-row tiles (2 heads each)
                for qt in range(2):
                    ps = psum.tile([128, 2 * TC], fp32, tag="big", bufs=3,
                                   name="ps_q")[:, 0:TC]
                    for k in range(16):
                        nc.tensor.matmul(ps[:], wq_sb[:, k, 128 * qt:128 * (qt + 1)],
                                         xt[:, k, :], start=(k == 0), stop=(k == 15))
                    rope(qT[qt][:, tsl], ps, ctq[:, 0, :], ctq[:, 1, :], 128)

                # K^T (rows 0:64) and V^T (rows 64:128) in one packed projection
                ps = psum.tile([128, 2 * TC], fp32, tag="big", bufs=3,
                               name="ps_kv")[:, 0:TC]
                for k in range(16):
                    nc.tensor.matmul(ps[:], wkv_sb[:, k, :], xt[:, k, :],
                                     start=(k == 0), stop=(k == 15))
                rope(k2[0:64, tsl], ps, ctk[:, 0, :], ctk[:, 1, :], 64)
                nc.scalar.copy(k2[64:128, tsl], k2[0:64, tsl])

                vt = work.tile([64, TC], bf16, tag="vt", name="vt")
                nc.scalar.copy(vt[:], ps[64:128, :])
                for j in range(TC // KB):
                    kbi = (TC // KB) * tc8 + j
                    pst = psum.tile([128, TC], bf16, tag="mm", bufs=2, name="ps_tr")
                    nc.tensor.transpose(pst[:, 0:64], vt[:, 128 * j:128 * (j + 1)],
                                        ident[0:64, 0:64])
                    nc.vector.tensor_copy(vatt[kbi][:, 0:D], pst[:, 0:64])

            # mask + ones-columns: only read from attention on; emitting them
            # here keeps the startup DMA queues clear for x/weights.
            nc.gpsimd.dma_start(msq[:], mq_d[:])
            for i in range(T // KB):
                nc.gpsimd.memset(vatt[i][:, D:D + 1], 1.0)

            # ---- wo prefetch: runs on DMA queues during attention ----
            # nt 0/1 get persistent tiles; nt 2/3 reuse the freed x-stream
            # slots (same [128, 16, 512] bf16 shape, x is done after phase 1).
            wo_r = wo.rearrange("(k p) n -> p k n", p=128)
            wo_sb = []
            w_eng = (nc.sync, nc.scalar, nc.gpsimd, nc.sync)
            for nt in range(4):
                if nt < 2:
                    t = persist.tile([128, 16, 512], bf16, tag=f"wo{nt}",
                                     name=f"wo{nt}")
                else:
                    t = stream.tile([128, 16, 512], bf16, tag="s", name=f"wo{nt}")
                w_eng[nt].dma_start(t[:], wo_r[:, :, 512 * nt:512 * (nt + 1)])
                wo_sb.append(t)

            # ================= Attention (head-pair parallel, causal) ==========
            a2a_in = [dram.tile([NC, 130, TL], bf16, tag=f"a2a_in{t}",
                                name=f"a2a_in{t}") for t in range(2)]
            a2a_out = [dram.tile([NC, 130, TL], bf16, tag=f"a2a_out{t}",
                                 name=f"a2a_out{t}") for t in range(2)]
            den_sb = [persist.tile([2 * NC, TC], bf16, tag=f"den{t}",
                                   name=f"den{t}") for t in range(2)]
            ao = {}
            mask_ctr = 0
            exp_acc = 0.0

            def attn_pair(pair):
                nonlocal mask_ctr, exp_acc
                qtile = qT[pair]
                for b in range(B):
                    for cq in range(4):
                        nkb = 4 * (cq + 1)
                        qs = S * b + TC * cq
                        j = 4 * b + cq          # destination core for this unit
                        psOa = psum.tile([128, 512], fp32, tag="mm", bufs=2,
                                         name="psOa")[0:D + 1, :]
                        psOb = psum.tile([128, 512], fp32, tag="mm", bufs=2,
                                         name="psOb")[0:D + 1, :]
                        for kb in range(nkb):
                            dj = kb - (nkb - 4)  # >=0: diagonal-band index
                            qoff = 128 * dj if (DIAG_TRIM and dj > 0) else 0
                            kpos = S * b + KB * kb
                            psS = psum.tile([128, 2 * TC], fp32, tag="big",
                                            bufs=3, name="psS")
                            ex = work.tile([128, 2 * TC], bf16, tag="ex", bufs=6,
                                           name="ex")
                            for h in range(2):
                                nc.tensor.matmul(
                                    psS[:, TC * h + qoff:TC * (h + 1)],
                                    k2[64 * h:64 * (h + 1), kpos:kpos + KB],
                                    qtile[64 * h:64 * (h + 1),
                                          qs + qoff:qs + TC],
                                    start=True, stop=True)
                            ps2 = psS.rearrange("p (h c) -> p h c", h=2)
                            ex2 = ex.rearrange("p (h c) -> p h c", h=2)
                            exi = ex.bitcast(i16).rearrange("p (h c) -> p h c", h=2)
                            exp_acc += DVE_EXP
                            if exp_acc >= 1.0:
                                exp_acc -= 1.0
                                nc.vector.tensor_scalar(
                                    exi[:, :, qoff:TC], ps2[:, :, qoff:TC],
                                    SCHR_A, SCHR_B,
                                    mybir.AluOpType.mult, mybir.AluOpType.add)
                            else:
                                nc.scalar.activation(
                                    ex2[:, :, qoff:TC], ps2[:, :, qoff:TC],
                                    mybir.ActivationFunctionType.Exp)
                            if dj >= 0:
                                # only the 128x128 diagonal square needs masking
                                meng = nc.gpsimd if (mask_ctr % 2 == 0) else nc.vector
                                mask_ctr += 1
                                meng.tensor_mul(ex2[:, :, qoff:qoff + 128],
                                                ex2[:, :, qoff:qoff + 128],
                                                msq[:, :, :])
                            for h, psO in ((0, psOa), (1, psOb)):
                                nc.tensor.matmul(
                                    psO[:, qoff:TC],
                                    vatt[(S // KB) * b + kb][:],
                                    ex2[:, h, qoff:TC],
                                    start=(kb == 0), stop=(kb == nkb - 1))
                        # drain: attn rows straight to a2a staging; den row to
                        # the local den staging tile (reciprocal'd pre-a2a)
                        for h, psO in ((0, psOa), (1, psOb)):
                            bounce = work.tile([D + 1, 512], bf16, tag="bounce",
                                               bufs=4, name="bounce")
                            if h == 0:
                                nc.scalar.copy(bounce[:], psO[:])
                            else:
                                nc.vector.tensor_copy(bounce[:], psO[:])
                            nc.sync.dma_start(
                                a2a_in[pair][j, 65 * h:65 * h + D, :],
                                bounce[0:D, :])
                            nc.sync.dma_start(
                                den_sb[pair][2 * j + h:2 * j + h + 1, :],
                                bounce[D:D + 1, :])

            i32 = mybir.dt.int32
            RMAGIC = 0x7EF127EA

            def recip_stage(pair):
                # 1/den on GpSimd (float bit-trick + 2 Newton steps) so neither
                # the Scalar nor Vector queue is touched; the reciprocals ride
                # the a2a in the payload's den rows (64/129 of each slot).
                ds = work.tile([2 * NC, TC], fp32, tag="rc_ds", bufs=2, name="ds")
                nc.gpsimd.tensor_copy(ds[:], den_sb[pair][:])      # bf16 -> f32
                bf = work.tile([2 * NC, TC], fp32, tag="rc_bf", bufs=2, name="bf")
                nc.gpsimd.tensor_copy(bf[:], ds.bitcast(i32)[:])   # bits as f32
                nc.gpsimd.tensor_scalar(bf[:], bf[:], -1.0, float(RMAGIC),
                                        mybir.AluOpType.mult,
                                        mybir.AluOpType.add)
                r0i = work.tile([2 * NC, TC], i32, tag="r0i", bufs=2, name="r0i")
                nc.gpsimd.tensor_copy(r0i[:], bf[:])               # back to bits
                r0 = r0i.bitcast(fp32)
                t = work.tile([2 * NC, TC], fp32, tag="rc_t", bufs=2, name="rc_t")
                for _ in range(2):
                    nc.gpsimd.tensor_mul(t[:], ds[:], r0[:])
                    nc.gpsimd.tensor_scalar(t[:], t[:], -1.0, 2.0,
                                            mybir.AluOpType.mult,
                                            mybir.AluOpType.add)
                    nc.gpsimd.tensor_mul(r0[:], r0[:], t[:])
                rb = work.tile([2 * NC, TC], bf16, tag="rcb", bufs=2, name="rcb")
                nc.gpsimd.tensor_copy(rb[:], r0[:])
                nc.gpsimd.dma_start(
                    a2a_in[pair].rearrange("a (x b) c -> (a x) b c", x=2)[:, 64, :],
                    rb[:])

            def a2a_post(pair):
                # payload den rows already hold 1/den (bf16): broadcast straight
                # from the a2a output in DRAM and multiply into the persistent
                # ao tile.  All DMA targets use the "bounce"-tag ring so the
                # scheduler sees a real dependency on late pair-1 work and
                # cannot queue these ahead of attention (a blocked queue head
                # would starve the whole machine while the a2a runs).
                a2a_v = a2a_out[pair].rearrange("a (x b) c -> (a x) b c", x=2)
                for r in range(NC):
                    kk = 2 * r + pair
                    tl_ = persist.tile([128, TL], bf16, tag=f"ao{kk}",
                                       name=f"ao{kk}")
                    w = work.tile([128, TL], bf16, tag="bounce", bufs=4,
                                  name="aow")
                    rb2 = work.tile([128, TL], bf16, tag="bounce", bufs=4,
                                    name="rb2")
                    beng = nc.gpsimd if (r % 2 == 0) else nc.sync
                    nc.sync.dma_start(w[0:64, :], a2a_out[pair][r, 0:64, :])
                    nc.sync.dma_start(w[64:128, :], a2a_out[pair][r, 65:129, :])
                    beng.dma_start(
                        rb2[0:64, :],
                        a2a_v[2 * r:2 * r + 1, 64, :].broadcast_to([64, TL]))
                    beng.dma_start(
                        rb2[64:128, :],
                        a2a_v[2 * r + 1:2 * r + 2, 64, :].broadcast_to([64, TL]))
                    nc.vector.tensor_mul(tl_[:], w[:], rb2[:])
                    ao[kk] = tl_

            def a2a_go(pair):
                nc.gpsimd.collective_compute(
                    "AllToAll", mybir.AluOpType.bypass,
                    replica_groups=[list(range(NC))],
                    ins=[a2a_in[pair].opt()], outs=[a2a_out[pair].opt()])

            # pair-0 post-collective work is emitted AFTER pair-1's attention
            # so the FIFO engine queues never make pair-1's attention wait on
            # the a2a (the ring-gated DMA targets enforce late placement).
            attn_pair(0)
            recip_stage(0)
            a2a_go(0)
            attn_pair(1)
            recip_stage(1)
            a2a_go(1)
            a2a_post(0)
            a2a_post(1)

            # ================= Phase 2: output projection =====================
            # kk-even (pair-0) contraction first: those matmuls only need the
            # pair-0 ao tiles, so the PE fills part of the a2a-1 wait.
            kk_order = [2 * r for r in range(8)] + [2 * r + 1 for r in range(8)]
            pg = 0
            for nt in range(4):
                for tt in range(TL // 128):
                    if pg % 5 < 3:
                        ps = psum.tile([128, 2 * TC], fp32, tag="big", bufs=3,
                                       name="ps_o")[:, 0:512]
                    else:
                        ps = psum.tile([128, 512], fp32, tag="mm", bufs=2,
                                       name="ps_o")
                    pg += 1
                    for ki, kk in enumerate(kk_order):
                        nc.tensor.matmul(ps[:], ao[kk][:, 128 * tt:128 * (tt + 1)],
                                         wo_sb[nt][:, kk, :],
                                         start=(ki == 0), stop=(ki == 15))
                    ob = work.tile([128, 512], fp32, tag="ob", bufs=2, name="ob")
                    if (nt + tt) % 2 == 0:
                        nc.scalar.copy(ob[:], ps[:])
                    else:
                        nc.vector.tensor_copy(ob[:], ps[:])
                    oeng = nc.gpsimd if (nt + tt) % 2 == 0 else nc.sync
                    oeng.dma_start(out_d[128 * tt:128 * (tt + 1),
                                         512 * nt:512 * (nt + 1)], ob[:])

    nc.compile()
    return nc


def _prep_inputs(x, cos, sin, wq, wk, wv, wo):
    x = np.asarray(x, F32)
    cos = np.asarray(cos, F32)
    sin = np.asarray(sin, F32)
    wq = np.asarray(wq, F32)
    wk = np.asarray(wk, F32)
    wv = np.asarray(wv, F32)
    wo = np.asarray(wo, F32)

    xT = np.ascontiguousarray(x.reshape(T, HID).T).astype(BF16)
    wo_b = wo.astype(BF16)

    pos = np.arange(T) % S
    sign = np.concatenate([-np.ones(D // 2, F32), np.ones(D // 2, F32)])
    ctk = np.ascontiguousarray(cos[pos].T)                      # [64, T]
    stk = np.ascontiguousarray((sin[pos] * sign).T)             # [64, T]
    ctk2 = np.ascontiguousarray(np.stack([ctk, stk], 1))        # [64, 2, T]
    scale = F32(1.0 / np.sqrt(D))
    ctq2 = np.ascontiguousarray(
        np.stack([np.concatenate([ctk, ctk], 0) * scale,
                  np.concatenate([stk, stk], 0) * scale], 1))   # [128, 2, T]

    # single 128x128 lower-tri mask, doubled for the two heads of a pair
    kl = np.arange(128)
    msq = (kl[None, :] >= kl[:, None]).astype(BF16)
    mq = np.ascontiguousarray(np.stack([msq, msq], 1))          # [128, 2, 128]

    in_maps = []
    for c in range(NC):
        wq_cc = np.ascontiguousarray(wq[:, c * LH * D:(c + 1) * LH * D]).astype(BF16)
        wkv_cc = np.concatenate(
            [wk[:, c * D:(c + 1) * D], wv[:, c * D:(c + 1) * D]], 1).astype(BF16)
        in_maps.append({
            "xT": xT, "wq_c": wq_cc, "wkv_c": wkv_cc, "wo": wo_b,
            "ctq": ctq2, "ctk": ctk2, "mq": mq,
        })
    return in_maps


def get_nc():
    if "nc" not in _CACHE:
        _CACHE["nc"] = _build()
    return _CACHE["nc"]


def run(in_maps, **kwargs):
    nc = get_nc()
    return run_bass_kernel_spmd(nc, in_maps, core_ids=list(range(NC)), **kwargs)


def kernel(x, cos, sin, wq, wk, wv, wo):
    in_maps = _prep_inputs(x, cos, sin, wq, wk, wv, wo)
    res = run(in_maps)
    out = np.empty((T, HID), F32)
    for c in range(NC):
        out[TL * c:TL * (c + 1)] = res.results[c]["out"]
    return out.reshape(B, S, HID)
